# revision 2
# baseline (speedup 1.0000x reference)
"""BottleNeck-MHSA (B=16, C=512, H=W=32, NH=8, DK=64) on 8 Trainium2 cores.

Sharding: pure data-parallel over batch (2 batches per core), no collectives.

Kernel design (per core):
- Weights pre-permuted host-side to head-major channel order c' = nh*64 + d.
- Rel-pos bias folded into the energy matmul via an augmented K=128
  contraction: Qaug rows = [qT(64) | ahT(32) | awT(32)], Kaug rows =
  [kT(64) | OneHotU(32) | OneHotW(32)].  S'^T[j,i] = energy^T + bias^T in one
  full-depth matmul (the DK=64 attention matmul would otherwise waste half
  the PE array).
- ahT/awT are content-dependent diagonal gathers of LHT = rel_h @ q and
  LWT = rel_w @ q; implemented as 32 rectangular SBUF->SBUF DMAs each
  (+ one DVE permute copy for awT since only one of the two can land
  x-major directly).
- Softmax skips max-subtraction (logits bounded ~+-6); exp on ACT with the
  1/sqrt(DK) scale fused into the ACTIVATE instruction; normalization
  deferred past the AV matmul: AV lhsT = [V | ones*64] so the denominator
  comes out replicated in rows 64-127 of the AV PSUM for free.
- exp scores and V in bf16 (fp32 PSUM accumulation), everything else fp32.
"""

from contextlib import ExitStack

import numpy as np

import concourse.bass as bass
import concourse.tile as tile
from concourse import bacc, mybir
from concourse.ap import AP
from concourse.bass_utils import run_bass_kernel_spmd

FP32 = mybir.dt.float32
BF16 = mybir.dt.bfloat16
Exp = mybir.ActivationFunctionType.Exp

B = 16
C = 512
N = 1024
NH = 8
DK = 64
HW = 32
NCORES = 8
NB = B // NCORES  # batches per core


def _build_body(ctx: ExitStack, tc: tile.TileContext, outs, ins, NB: int):
    nc = tc.nc
    x_in, wq_in, wk_in, wv_in, wo_in, oh_in, rh_in, rw_in, bo_in = ins
    y_out = outs[0]

    consts = ctx.enter_context(tc.tile_pool(name="consts", bufs=1))
    persist = ctx.enter_context(tc.tile_pool(name="persist", bufs=1))
    work = ctx.enter_context(tc.tile_pool(name="work", bufs=2))
    expp = ctx.enter_context(tc.tile_pool(name="expp", bufs=9))
    psum = ctx.enter_context(tc.tile_pool(name="psum", bufs=3, space="PSUM"))
    psum_av = ctx.enter_context(tc.tile_pool(name="psum_av", bufs=1, space="PSUM"))

    # ---------------- constants ----------------
    w_t = {}
    for nm, src in (("wq", wq_in), ("wk", wk_in), ("wv", wv_in), ("wo", wo_in)):
        for kc in range(4):
            t = consts.tile([128, C], FP32, tag=f"{nm}{kc}", name=f"{nm}{kc}")
            nc.sync.dma_start(t[:], src[kc * 128 : (kc + 1) * 128, :])
            w_t[nm, kc] = t
    oh_t = consts.tile([64, N], FP32, tag="onehot", name="onehot")
    nc.sync.dma_start(oh_t[:], oh_in[:])
    rh_t = consts.tile([128, 126], FP32, tag="relh2", name="relh2")
    nc.sync.dma_start(rh_t[:], rh_in[:])
    rw_t = consts.tile([128, 126], FP32, tag="relw2", name="relw2")
    nc.sync.dma_start(rw_t[:], rw_in[:])
    bo_t = consts.tile([128, 4], FP32, tag="bo", name="bo")
    nc.sync.dma_start(bo_t[:], bo_in[:].rearrange("(c p) one -> p (c one)", p=128))

    # ---------------- persistent work tiles ----------------
    x_t = [persist.tile([128, N], FP32, tag=f"x{kc}", name=f"x{kc}") for kc in range(4)]
    qaug = [persist.tile([128, N], FP32, tag=f"qaug{h}", name=f"qaug{h}") for h in range(NH)]
    kaug = [persist.tile([128, N], FP32, tag=f"kaug{h}", name=f"kaug{h}") for h in range(NH)]
    vaug = [
        [persist.tile([128, 128], BF16, tag=f"vaug{h}_{jb}", name=f"vaug{h}_{jb}") for jb in range(8)]
        for h in range(NH)
    ]
    oin = [qaug[2 * kc] for kc in range(4)]  # reuse: qaug[2kc] dead after S^T of head 2kc

    for h in range(NH):
        for jb in range(8):
            nc.vector.memset(vaug[h][jb][:, 64:128], 1.0)

    for b in range(NB):
        for kc in range(4):
            nc.sync.dma_start(x_t[kc][:], x_in[b, kc * 128 : (kc + 1) * 128, :])

        # ---------------- Q projection + rel-pos prep ----------------
        for mc in range(4):  # head pair (2mc, 2mc+1)
            pq = psum.tile([128, N], FP32, tag="mm", name="mm")
            for kc in range(4):
                for nn in range(2):
                    nc.tensor.matmul(
                        pq[:, nn * 512 : (nn + 1) * 512],
                        w_t["wq", kc][:, mc * 128 : (mc + 1) * 128],
                        x_t[kc][:, nn * 512 : (nn + 1) * 512],
                        start=(kc == 0),
                        stop=(kc == 3),
                    )
            qpair = work.tile([128, N], FP32, tag="qpair", name="qpair", bufs=1)
            nc.vector.tensor_copy(qpair[:], pq[:])

            # LHT pair: rows 0-62 head 2mc, 63-125 head 2mc+1 (x-major cols)
            plh = psum.tile([126, N], FP32, tag="mm", name="mm")
            for nn in range(2):
                nc.tensor.matmul(
                    plh[:, nn * 512 : (nn + 1) * 512],
                    rh_t[:],
                    qpair[:, nn * 512 : (nn + 1) * 512],
                    start=True,
                    stop=True,
                )
            lht = work.tile([126, N], FP32, tag="lht", name="lht", bufs=1)
            nc.vector.tensor_copy(lht[:], plh[:])

            # LWT pair with y-major columns: col 32y+x reads qpair col 32x+y
            plw = psum.tile([126, N], FP32, tag="mm", name="mm")
            qp = qpair[:]
            for nn in range(2):
                ym = AP(qp.tensor, qp.offset + nn * 16, [[1024, 128], [1, 16], [32, 32]])
                nc.tensor.matmul(
                    plw[:, nn * 512 : (nn + 1) * 512],
                    rw_t[:],
                    ym,
                    start=True,
                    stop=True,
                )
            lwt = work.tile([126, N], FP32, tag="lwt", name="lwt", bufs=1)
            nc.vector.tensor_copy(lwt[:], plw[:])

            for hh in range(2):
                h = 2 * mc + hh
                nc.vector.tensor_copy(qaug[h][0:64, :], qpair[hh * 64 : hh * 64 + 64, :])
                # ahT rows 64-95: qaug[h][64+u, 32x+y] = lht[hh*63 + u+31-x, 32x+y]
                for xx in range(32):
                    base = hh * 63 + 31 - xx
                    nc.sync.dma_start(
                        qaug[h][64:96, 32 * xx : 32 * xx + 32],
                        lht[base : base + 32, 32 * xx : 32 * xx + 32],
                    )
                # awT y-major scratch: awym[w, 32y+x] = lwt[hh*63 + w+31-y, 32y+x]
                awym = work.tile([32, N], FP32, tag="awym", name="awym")
                for yy in range(32):
                    base = hh * 63 + 31 - yy
                    nc.sync.dma_start(
                        awym[:, 32 * yy : 32 * yy + 32],
                        lwt[base : base + 32, 32 * yy : 32 * yy + 32],
                    )
                # permute y-major -> x-major into qaug rows 96-127
                aw = awym[:]
                src = AP(aw.tensor, aw.offset, [[1024, 32], [1, 32], [32, 32]])
                nc.vector.tensor_copy(
                    qaug[h][96:128, :].rearrange("p (a b) -> p a b", a=32), src
                )

        # ---------------- K projection ----------------
        for mc in range(4):
            pk = psum.tile([128, N], FP32, tag="mm", name="mm")
            for kc in range(4):
                for nn in range(2):
                    nc.tensor.matmul(
                        pk[:, nn * 512 : (nn + 1) * 512],
                        w_t["wk", kc][:, mc * 128 : (mc + 1) * 128],
                        x_t[kc][:, nn * 512 : (nn + 1) * 512],
                        start=(kc == 0),
                        stop=(kc == 3),
                    )
            for hh in range(2):
                h = 2 * mc + hh
                nc.vector.tensor_copy(kaug[h][0:64, :], pk[hh * 64 : hh * 64 + 64, :])
                nc.vector.tensor_copy(kaug[h][64:128, :], oh_t[:])

        # ---------------- V projection (n-block orientation) ----------------
        for nb in range(8):
            pv = psum.tile([128, 512], FP32, tag="mm", name="mm")
            for kc in range(4):
                nc.tensor.matmul(
                    pv[:],
                    x_t[kc][:, nb * 128 : (nb + 1) * 128],
                    w_t["wv", kc][:],
                    start=(kc == 0),
                    stop=(kc == 3),
                )
            for h in range(NH):
                nc.vector.tensor_copy(vaug[h][nb][:, 0:64], pv[:, h * 64 : h * 64 + 64])

        # ---------------- attention per head ----------------
        for h in range(NH):
            est = [expp.tile([128, N], BF16, tag="expst", name="expst") for jb in range(8)]
            for jb in range(8):
                pst = psum.tile([128, N], FP32, tag="mm", name="mm")
                for nn in range(2):
                    nc.tensor.matmul(
                        pst[:, nn * 512 : (nn + 1) * 512],
                        kaug[h][:, jb * 128 : (jb + 1) * 128],
                        qaug[h][:, nn * 512 : (nn + 1) * 512],
                        start=True,
                        stop=True,
                    )
                nc.scalar.activation(est[jb][:], pst[:], Exp, bias=0.0, scale=0.125)
            pav = psum_av.tile([128, N], FP32, tag="av", name="av")
            for jb in range(8):
                for nn in range(2):
                    nc.tensor.matmul(
                        pav[:, nn * 512 : (nn + 1) * 512],
                        vaug[h][jb][:],
                        est[jb][:, nn * 512 : (nn + 1) * 512],
                        start=(jb == 0),
                        stop=(jb == 7),
                    )
            recip = work.tile([64, N], FP32, tag="recip", name="recip")
            nc.vector.reciprocal(recip[:], pav[64:128, :])
            nc.vector.tensor_mul(
                oin[h // 2][(h % 2) * 64 : (h % 2) * 64 + 64, :], pav[0:64, :], recip[:]
            )

        # ---------------- O projection + bias ----------------
        for mc in range(4):
            po = psum.tile([128, N], FP32, tag="mm", name="mm")
            for kc in range(4):
                for nn in range(2):
                    nc.tensor.matmul(
                        po[:, nn * 512 : (nn + 1) * 512],
                        w_t["wo", kc][:, mc * 128 : (mc + 1) * 128],
                        oin[kc][:, nn * 512 : (nn + 1) * 512],
                        start=(kc == 0),
                        stop=(kc == 3),
                    )
            oo = work.tile([128, N], FP32, tag="oout", name="oout")
            nc.vector.tensor_add(oo[:], po[:], bo_t[:, mc : mc + 1].broadcast_to((128, N)))
            nc.sync.dma_start(y_out[b, mc * 128 : (mc + 1) * 128, :], oo[:])


def _host_prep(w_q, w_k, w_v, w_o, b_o, rel_h, rel_w):
    perm = np.array([(c % 64) * 8 + c // 64 for c in range(C)])  # c' -> orig c
    oh = np.zeros((64, N), np.float32)
    j = np.arange(N)
    oh[j // HW, j] = 1.0
    oh[32 + j % HW, j] = 1.0
    rh2 = np.zeros((128, 126), np.float32)
    rh2[0:64, 0:63] = rel_h.T
    rh2[64:128, 63:126] = rel_h.T
    rw2 = np.zeros((128, 126), np.float32)
    rw2[0:64, 0:63] = rel_w.T
    rw2[64:128, 63:126] = rel_w.T
    return dict(
        wq=np.ascontiguousarray(w_q[perm, :].T, dtype=np.float32),
        wk=np.ascontiguousarray(w_k[perm, :].T, dtype=np.float32),
        wv=np.ascontiguousarray(w_v[perm, :].T, dtype=np.float32),
        wo=np.ascontiguousarray(w_o.T, dtype=np.float32),
        onehot=oh,
        relh2=rh2,
        relw2=rw2,
        bo=np.ascontiguousarray(b_o.reshape(C, 1), dtype=np.float32),
    )


_CACHE = {}


def _build_program():
    if "nc" in _CACHE:
        return _CACHE["nc"], _CACHE["names"]
    nc = bacc.Bacc("TRN2", target_bir_lowering=False, debug=False, num_devices=NCORES)
    specs = [
        ("x", (NB, C, N), FP32),
        ("wq", (C, C), FP32),
        ("wk", (C, C), FP32),
        ("wv", (C, C), FP32),
        ("wo", (C, C), FP32),
        ("onehot", (64, N), FP32),
        ("relh2", (128, 126), FP32),
        ("relw2", (128, 126), FP32),
        ("bo", (C, 1), FP32),
    ]
    in_aps = [nc.dram_tensor(nm, list(shape), dt, kind="ExternalInput").ap() for nm, shape, dt in specs]
    out_ap = nc.dram_tensor("y", [NB, C, N], FP32, kind="ExternalOutput").ap()
    with tile.TileContext(nc) as tc:
        with ExitStack() as ctx:
            _build_body(ctx, tc, [out_ap], in_aps, NB)
    nc.compile()
    _CACHE["nc"] = nc
    _CACHE["names"] = [s[0] for s in specs]
    return nc, _CACHE["names"]


def _run(inputs, trace=False, tmpdir=None):
    x = np.asarray(inputs["x"], dtype=np.float32)
    cst = _host_prep(
        np.asarray(inputs["w_q"], np.float32),
        np.asarray(inputs["w_k"], np.float32),
        np.asarray(inputs["w_v"], np.float32),
        np.asarray(inputs["w_o"], np.float32),
        np.asarray(inputs["b_o"], np.float32),
        np.asarray(inputs["rel_h"], np.float32),
        np.asarray(inputs["rel_w"], np.float32),
    )
    nc, _ = _build_program()
    in_maps = []
    for c in range(NCORES):
        m = dict(cst)
        m["x"] = np.ascontiguousarray(x[c * NB : (c + 1) * NB].reshape(NB, C, N))
        in_maps.append(m)
    res = run_bass_kernel_spmd(
        nc, in_maps, core_ids=list(range(NCORES)), trace=trace, tmpdir=tmpdir
    )
    out = np.empty((B, C, HW, HW), np.float32)
    for c in range(NCORES):
        out[c * NB : (c + 1) * NB] = res.results[c]["y"].reshape(NB, C, HW, HW)
    return out, res


def kernel(**inputs):
    out, _ = _run(inputs, trace=False)
    return out
